# revision 19
# baseline (speedup 1.0000x reference)
"""Trainium2 Bass kernel for MoRAttention (sparse selective-KV GQA attention).

Math: the reference's argsort/gather of active keys == dense attention with
mask = active[k] & (pos[k] <= pos[q]) (softmax is permutation invariant along
keys; q columns are independent). We permute the q axis on the host to
actives-first order, so the first SA columns of the permuted hidden state ARE
the compacted keys: K/V projections read a prefix of the same SBUF tensor the
Q projection reads — no separate gathered-x input. The host inverse-permutes
the output columns.

Causal structure over sorted keys/queries lets us statically skip dead
(k-chunk, q-span) tiles; tiles that are computed but not fully valid get a
multiplicative mask precomputed on the host.

Sharding: 8 cores = 2 batches x 4 kv-groups. Core (b, g) computes q-heads
[4g, 4g+4) + kv-head g of batch b, producing a partial o_proj output
out^T [D, S]; the host sums the 4 partials per batch.

All matmul operands bf16 (same PE rate as f32r, half the DMA/SBUF traffic),
f32 PSUM.

Schedule: inputs stream as ~20 wide DMAs (HWDGE trigger cost ~0.6us each, so
few+large beats many+small) ordered to match consumption; the K, V and Q0
projection chains interleave per-chunk behind the xs stream so the PE starts
as soon as chunk 0 lands. V is computed directly in [key, hd] layout
(lhsT = xs key-block) so no psum transpose is needed. RoPE half-swaps use
partition-shifted DVE multiplies (no SBUF-SBUF DMA). Late-consumed bulk
(wq for heads 2-3, wo) is held behind a dummy dependency so its HBM traffic
does not compete with the critical stream. Attention heads pipeline with the
next head's Q-projection as PE filler; o_proj drains per-half for a short
tail.
"""

import numpy as np

S, D, HD = 1024, 2048, 128
NH = 4           # q heads per core
DC = D // 128    # contraction chunks
SCALE = HD ** -0.5

TRACE = False
DEBUG_TAPS = False
LAST_EXEC_NS = None
LAST_RESULTS = None

_NC_CACHE = {}


def _build_nc(meta):
    import concourse.mybir as mybir
    from concourse import bacc
    from concourse.tile import TileContext
    from contextlib import ExitStack

    SAC, windows, mask_runs, nstr = meta
    SA = SAC * 128
    NSTR = max(1, nstr)
    runs_by_kc = {}
    for (kc, qt0, n, idx0) in mask_runs:
        runs_by_kc.setdefault(kc, []).append((qt0, n, idx0))

    f32 = mybir.dt.float32
    bf16 = mybir.dt.bfloat16
    Exp = mybir.ActivationFunctionType.Exp

    nc = bacc.Bacc("TRN2", target_bir_lowering=False, debug=False)

    xs_d = nc.dram_tensor("xs", [128, DC * S], bf16, kind="ExternalInput")
    wq_d = nc.dram_tensor("wq", [128, NH * DC * 128], bf16, kind="ExternalInput")
    wk_d = nc.dram_tensor("wk", [128, DC * 128], bf16, kind="ExternalInput")
    wv_d = nc.dram_tensor("wv", [128, DC * 128], bf16, kind="ExternalInput")
    wo_d = nc.dram_tensor("wo", [128, NH * D], bf16, kind="ExternalInput")
    cs_d = nc.dram_tensor("cs", [128, 2 * S], bf16, kind="ExternalInput")
    mk_d = nc.dram_tensor("mk", [128, NSTR * 128], bf16, kind="ExternalInput")
    out_d = nc.dram_tensor("out", [128, DC * S], bf16, kind="ExternalOutput")
    if DEBUG_TAPS:
        dbg_q = [nc.dram_tensor(f"dbg_q{h}", [128, S], bf16, kind="ExternalOutput")
                 for h in range(NH)]
        dbg_kT = nc.dram_tensor("dbg_kT", [128, SA], bf16, kind="ExternalOutput")
        dbg_vn = nc.dram_tensor("dbg_vn", [128, SA], bf16, kind="ExternalOutput")
        dbg_at = [nc.dram_tensor(f"dbg_at{h}", [128, S], bf16, kind="ExternalOutput")
                  for h in range(NH)]
        dbg_c = nc.dram_tensor("dbg_c", [128, S], mybir.dt.float32, kind="ExternalOutput")
        dbg_o = nc.dram_tensor("dbg_o", [128, S], mybir.dt.float32, kind="ExternalOutput")
        dbg_p = nc.dram_tensor("dbg_p", [128, S], bf16, kind="ExternalOutput")

    with TileContext(nc) as tc, ExitStack() as ctx:
        singles = ctx.enter_context(tc.tile_pool(name="singles", bufs=1))
        persist = ctx.enter_context(tc.tile_pool(name="persist", bufs=1))

        ones_tmp = singles.tile([128, 128], f32)
        nc.vector.memset(ones_tmp, 1.0)
        ones128 = singles.tile([128, 128], bf16)
        nc.vector.tensor_copy(ones128, ones_tmp)

        xs_sb = persist.tile([128, DC * S], bf16, tag="xs")
        wq_sb = persist.tile([128, NH * DC * 128], bf16, tag="wq")
        wk_sb = persist.tile([128, DC * 128], bf16, tag="wk")
        wv_sb = persist.tile([128, DC * 128], bf16, tag="wv")
        wo_sb = persist.tile([128, NH * D], bf16, tag="wo")
        cs_sb = persist.tile([128, 2 * S], bf16, tag="cs")
        mk_sb = persist.tile([128, NSTR * 128], bf16, tag="mk")
        cq_sb = cs_sb[:, 0:S]
        sq_sb = cs_sb[:, S:2 * S]

        kT = persist.tile([128, SA], bf16, tag="kT")
        vn = persist.tile([128, SA], bf16, tag="vn")
        qT = [persist.tile([128, S], bf16, tag=f"qT{h}", name=f"qT{h}") for h in range(NH)]
        attn = [persist.tile([128, S], bf16, tag=f"attn{h}", name=f"attn{h}") for h in range(NH)]

        # ---- input DMAs: few, wide transfers (trigger cost ~0.6us each on
        # the issuing queue dominates small DMAs). Emission order == issue
        # order == consumption order. wq is head-major so each head's chain
        # depends on one transfer. wq2/wq3/wo are gated behind a dummy read
        # of qT[0] on the gpsimd queue so their bytes don't compete with the
        # critical K/V/Q0 stream for HBM bandwidth.
        hw = [nc.sync, nc.scalar]
        WQH = DC * 128  # per-head wq span
        # First arrivals gate the first matmuls, and cold DMA runs well below
        # steady bandwidth — keep the gating pieces tiny: wk/wv split so the
        # first 4 chunks' slices land first, xs chunks 0-3 split in halves.
        nc.scalar.dma_start(out=wk_sb[:, 0:512], in_=wk_d[:, 0:512])
        nc.scalar.dma_start(out=wv_sb[:, 0:512], in_=wv_d[:, 0:512])
        nc.scalar.dma_start(out=wk_sb[:, 512:], in_=wk_d[:, 512:])
        nc.scalar.dma_start(out=wv_sb[:, 512:], in_=wv_d[:, 512:])
        # wq0 before the xs stream: the in-order Tensor queue has Q0 matmuls
        # interleaved behind K/V, so a late wq0 would head-of-line block them
        nc.scalar.dma_start(out=wq_sb[:, 0:WQH], in_=wq_d[:, 0:WQH])
        for dc in range(DC):
            if dc < 4:
                for (a, b) in ((0, 512), (512, S)):
                    hw[dc % 2].dma_start(
                        out=xs_sb[:, dc * S + a: dc * S + b],
                        in_=xs_d[:, dc * S + a: dc * S + b],
                    )
            else:
                hw[dc % 2].dma_start(
                    out=xs_sb[:, dc * S:(dc + 1) * S], in_=xs_d[:, dc * S:(dc + 1) * S]
                )
            if dc == 8:
                nc.sync.dma_start(out=cs_sb, in_=cs_d[:, :])
        nc.sync.dma_start(out=wq_sb[:, WQH:2 * WQH], in_=wq_d[:, WQH:2 * WQH])
        nc.sync.dma_start(out=mk_sb, in_=mk_d[:, :])

        def rope(psum, qs, w, dst, pool):
            # dst = psum*cos + rot_half(psum)*sin, via partition-shifted DVE
            # muls (sq rows are pre-arranged + sign-flipped on the host so
            # the shifted product IS rot_half(psum)*sin).
            cos_t = cq_sb[:, qs:qs + w]
            sin_t = sq_sb[:, qs:qs + w]
            pc = pool.tile([128, w], bf16, tag=f"ropec{w}")
            pw = pool.tile([128, w], bf16, tag=f"ropew{w}")
            nc.vector.tensor_mul(pc, psum, cos_t)
            nc.vector.tensor_mul(pw[0:64, :], psum[64:128, :], sin_t[64:128, :])
            nc.vector.tensor_mul(pw[64:128, :], psum[0:64, :], sin_t[0:64, :])
            nc.vector.tensor_add(dst, pc, pw)

        # ================= Stream phase: K, V, Q0 behind the xs stream ====
        with tc.tile_pool(name="pq", bufs=2, space="PSUM") as pq, \
             tc.tile_pool(name="ropeq", bufs=2) as ropeq:
            with tc.tile_pool(name="pkv", bufs=2, space="PSUM") as pkv:
                psum_k = pkv.tile([128, SA], f32, tag="pkv", name="psum_k")
                vn_ps = pkv.tile([128, SA], f32, tag="pkv", name="vn_ps")
                psq0 = pq.tile([128, 512], f32, tag="pq", name="pq0_0")
                psq1 = pq.tile([128, 512], f32, tag="pq", name="pq0_512")

                def q0_mm(dc):
                    for qs, psq in ((0, psq0), (512, psq1)):
                        nc.tensor.matmul(
                            psq,
                            lhsT=wq_sb[:, dc * 128:(dc + 1) * 128],
                            rhs=xs_sb[:, dc * S + qs: dc * S + qs + 512],
                            start=(dc == 0), stop=(dc == DC - 1),
                        )

                LAG = 2
                for dc in range(DC):
                    for (c0, c1) in ((0, 512), (512, SA)):
                        nc.tensor.matmul(
                            psum_k[:, c0:c1],
                            lhsT=wk_sb[:, dc * 128:(dc + 1) * 128],
                            rhs=xs_sb[:, dc * S + c0: dc * S + c1],
                            start=(dc == 0), stop=(dc == DC - 1),
                        )
                    for kc in range(SAC):
                        # start=True clears the has_written bits of the WHOLE
                        # psum bank (4 chunks of 128 f32 per bank): only the
                        # first chunk in each bank may carry it, the others'
                        # first write then overwrites (has_written clear).
                        nc.tensor.matmul(
                            vn_ps[:, kc * 128:(kc + 1) * 128],
                            lhsT=xs_sb[:, dc * S + kc * 128: dc * S + (kc + 1) * 128],
                            rhs=wv_sb[:, dc * 128:(dc + 1) * 128],
                            start=(dc == 0 and kc % 4 == 0), stop=(dc == DC - 1),
                            skip_group_check=True,
                        )
                    if dc >= LAG:
                        q0_mm(dc - LAG)
                for dc in range(DC - LAG, DC):
                    q0_mm(dc)

                # K-ropes first: psum_k[:, 0:512] finalizes at K(dc15)'s first
                # matmul, so kT chunks 0-3 rope while the V/Q0 tail still
                # streams; Q0's ropes (which gate B0) follow as psq finalizes.
                for kc in range(SAC):
                    c0, c1 = kc * 128, (kc + 1) * 128
                    rope(psum_k[:, c0:c1], c0, 128, kT[:, c0:c1], ropeq)
                    nc.scalar.copy(vn[:, c0:c1], vn_ps[:, c0:c1])
                rope(psq0, 0, 512, qT[0][:, 0:512], ropeq)
                rope(psq1, 512, 512, qT[0][:, 512:], ropeq)

            # delayed bulk loads: their HBM traffic must not compete with the
            # critical stream. The scheduler reorders freely absent deps, so
            # gate each DMA with a real WAW hazard: a tiny copy (gated on
            # qT[0], ready at stream end) into the DMA's destination region.
            gate = qT[0][:, 0:2]
            nc.gpsimd.tensor_copy(wq_sb[:, 2 * WQH:2 * WQH + 2], gate)
            nc.gpsimd.dma_start(out=wq_sb[:, 2 * WQH:3 * WQH], in_=wq_d[:, 2 * WQH:3 * WQH])
            nc.gpsimd.tensor_copy(wq_sb[:, 3 * WQH:3 * WQH + 2], gate)
            nc.gpsimd.dma_start(out=wq_sb[:, 3 * WQH:4 * WQH], in_=wq_d[:, 3 * WQH:4 * WQH])
            nc.gpsimd.tensor_copy(wo_sb[:, 0:2], gate)
            nc.gpsimd.dma_start(out=wo_sb, in_=wo_d[:, :])

            # -------- attention heads + next-head Q chains, pipelined ------
            with tc.tile_pool(name="ps", bufs=2, space="PSUM") as ps_p, \
                 tc.tile_pool(name="po", bufs=1, space="PSUM") as po_p, \
                 tc.tile_pool(name="pc", bufs=1, space="PSUM") as pc_p, \
                 tc.tile_pool(name="ppool", bufs=2) as ppool, \
                 tc.tile_pool(name="rpool", bufs=2) as rpool:

                def q_chain_emit(h):
                    """One Q-projection matmul per next(); rope emitted as
                    each half completes so it overlaps the stream."""
                    for qs in (0, 512):
                        psq = pq.tile([128, 512], f32, tag="pq", name=f"pq{h}_{qs}")
                        for dc in range(DC):
                            nc.tensor.matmul(
                                psq,
                                lhsT=wq_sb[:, (h * DC + dc) * 128:(h * DC + dc + 1) * 128],
                                rhs=xs_sb[:, dc * S + qs: dc * S + qs + 512],
                                start=(dc == 0), stop=(dc == DC - 1),
                            )
                            yield None
                        rope(psq, qs, 512, qT[h][:, qs:qs + 512], ropeq)

                # last key-chunk whose windows touch the active q-half: after
                # its reduce, psum_o/psum_c[:, 0:512] are final and half-0 of
                # the normalize can overlap the remaining chunks' matmuls
                K_LAST_ACT = max(
                    (kc for kc in range(SAC)
                     if any(s0 < 512 for (s0, s1) in windows[kc])),
                    default=SAC - 1,
                )

                def b_norm_half(h, psum_o, psum_c, qs):
                    rb = rpool.tile([128, 512], f32, tag="rb", name=f"rb{h}_{qs}")
                    nc.vector.reciprocal_approx_fast(rb, psum_c[:, qs:qs + 512])
                    nc.vector.tensor_mul(
                        attn[h][:, qs:qs + 512], psum_o[:, qs:qs + 512], rb
                    )

                def b_head(h, psum_o, psum_c, filler):
                    def fill(n):
                        for _ in range(n):
                            next(filler, None)

                    def scores_exp(kc):
                        p_sb = ppool.tile([128, S], bf16, tag="p_sb", name=f"p{h}_{kc}")
                        for (s0, s1) in windows[kc]:
                            psum_s = ps_p.tile([128, 512], f32, tag="ps", name=f"ps{h}_{kc}_{s0}")
                            nc.tensor.matmul(
                                psum_s[:, 0:s1 - s0],
                                lhsT=kT[:, kc * 128:(kc + 1) * 128],
                                rhs=qT[h][:, s0:s1],
                                start=True, stop=True,
                            )
                            fill(3)
                            nc.scalar.activation(
                                p_sb[:, s0:s1], psum_s[:, 0:s1 - s0], Exp, scale=SCALE
                            )
                        for (qt0, n, idx0) in runs_by_kc.get(kc, ()):
                            nc.vector.tensor_mul(
                                p_sb[:, qt0 * 128:(qt0 + n) * 128],
                                p_sb[:, qt0 * 128:(qt0 + n) * 128],
                                mk_sb[:, idx0 * 128:(idx0 + n) * 128],
                            )
                        return p_sb

                    def reduce_chunk(kc, p_sb):
                        # kc=0 zeroes the full psum (it covers both halves);
                        # later chunks accumulate arbitrary 256-aligned
                        # sub-windows, so the emission-time group check is
                        # skipped (stop is simulator-only metadata).
                        start = (kc == 0)
                        stop = (kc == SAC - 1)
                        for (s0, s1) in windows[kc]:
                            nc.tensor.matmul(
                                psum_c[:, s0:s1], lhsT=ones128,
                                rhs=p_sb[:, s0:s1], start=start, stop=stop,
                                skip_group_check=True,
                            )
                            fill(2)
                            nc.tensor.matmul(
                                psum_o[:, s0:s1],
                                lhsT=vn[:, kc * 128:(kc + 1) * 128],
                                rhs=p_sb[:, s0:s1], start=start, stop=stop,
                                skip_group_check=True,
                            )
                            fill(2)

                    prev = None
                    fill(10)
                    for kc in range(SAC):
                        p_sb = scores_exp(kc)
                        if DEBUG_TAPS and h == 0 and kc == 0:
                            nc.gpsimd.dma_start(out=dbg_p[:, :], in_=p_sb)
                        fill(3)
                        if prev is not None:
                            reduce_chunk(prev[0], prev[1])
                            if prev[0] == K_LAST_ACT:
                                b_norm_half(h, psum_o, psum_c, 0)
                        prev = (kc, p_sb)
                    reduce_chunk(prev[0], prev[1])
                    if prev[0] == K_LAST_ACT:
                        b_norm_half(h, psum_o, psum_c, 0)
                    fill(64)  # drain any remaining interleaved Q matmuls

                for h in range(NH):
                    psum_o = po_p.tile([128, S], f32, tag="po", name=f"po{h}")
                    psum_c = pc_p.tile([128, S], f32, tag="pc", name=f"pc{h}")
                    filler = q_chain_emit(h + 1) if h + 1 < NH else iter(())
                    b_head(h, psum_o, psum_c, filler)
                    if DEBUG_TAPS and h == 0:
                        dbg_ct = rpool.tile([128, S], f32, tag="dbgc")
                        dbg_ot = rpool.tile([128, S], f32, tag="dbgo")
                        nc.vector.tensor_copy(dbg_ct, psum_c)
                        nc.vector.tensor_copy(dbg_ot, psum_o)
                        nc.gpsimd.dma_start(out=dbg_c[:, :], in_=dbg_ct)
                        nc.gpsimd.dma_start(out=dbg_o[:, :], in_=dbg_ot)
                    b_norm_half(h, psum_o, psum_c, 512)

        if DEBUG_TAPS:
            for h in range(NH):
                nc.sync.dma_start(out=dbg_q[h][:, :], in_=qT[h])
                nc.sync.dma_start(out=dbg_at[h][:, :], in_=attn[h])
            nc.sync.dma_start(out=dbg_kT[:, :], in_=kT)
            nc.sync.dma_start(out=dbg_vn[:, :], in_=vn)

        # ================= Phase C: out^T = wo^T @ attn =================
        with tc.tile_pool(name="poc", bufs=2, space="PSUM") as poc, \
             tc.tile_pool(name="outp", bufs=3) as outp:
            def copy_piece(use_scalar, dst, src):
                if use_scalar:
                    nc.scalar.copy(dst, src)
                else:
                    nc.vector.tensor_copy(dst, src)

            for dc in range(DC):
                oc = poc.tile([128, S], f32, tag="oc", name=f"oc{dc}")
                osb = outp.tile([128, S], bf16, tag="osb", name=f"osb{dc}")
                last = dc == DC - 1
                for qs in (0, 512):
                    for h in range(NH):
                        nc.tensor.matmul(
                            oc[:, qs:qs + 512],
                            lhsT=wo_sb[:, h * D + dc * 128: h * D + (dc + 1) * 128],
                            rhs=attn[h][:, qs:qs + 512],
                            start=(h == 0), stop=(h == NH - 1),
                        )
                    if last and qs == 512:
                        # split the very last piece across both copy engines +
                        # both DMA queues for the shortest drain
                        copy_piece(True, osb[:, 512:768], oc[:, 512:768])
                        copy_piece(False, osb[:, 768:1024], oc[:, 768:1024])
                        nc.sync.dma_start(
                            out=out_d[:, dc * S + 512: dc * S + 768],
                            in_=osb[:, 512:768],
                        )
                        nc.gpsimd.dma_start(
                            out=out_d[:, dc * S + 768:(dc + 1) * S],
                            in_=osb[:, 768:1024],
                        )
                    else:
                        copy_piece((dc + (qs > 0)) % 2 == 0,
                                   osb[:, qs:qs + 512], oc[:, qs:qs + 512])
                        if last:
                            nc.gpsimd.dma_start(
                                out=out_d[:, dc * S: dc * S + 512],
                                in_=osb[:, 0:512],
                            )
                if not last:
                    (nc.sync if dc % 2 == 0 else nc.gpsimd).dma_start(
                        out=out_d[:, dc * S:(dc + 1) * S], in_=osb
                    )

    nc.compile()
    return nc


def _get_nc(meta):
    if meta not in _NC_CACHE:
        _NC_CACHE[meta] = _build_nc(meta)
    return _NC_CACHE[meta]


def _host_prep(hidden_states, cos, sin, wq, wk, wv, wo, position_ids, active_mask):
    import ml_dtypes

    bf16 = ml_dtypes.bfloat16
    hs = np.asarray(hidden_states, dtype=np.float32)
    cos = np.asarray(cos, dtype=np.float32)
    sin = np.asarray(sin, dtype=np.float32)
    wq = np.asarray(wq, dtype=np.float32)
    wk = np.asarray(wk, dtype=np.float32)
    wv = np.asarray(wv, dtype=np.float32)
    wo = np.asarray(wo, dtype=np.float32)
    pos = np.asarray(position_ids).astype(np.int64)
    am = np.asarray(active_mask).astype(bool)
    B = hs.shape[0]
    assert B == 2 and hs.shape[1] == S and hs.shape[2] == D

    ar = np.arange(S)
    perms, pos_sels, nacts = [], [], []
    for b in range(B):
        # actives-first stable order == full q permutation; its prefix is the
        # compacted-key order
        perm = np.argsort(np.where(am[b], ar, ar + S), kind="stable")
        nact = int(am[b].sum())
        perms.append(perm)
        pos_sels.append(pos[b][perm[:nact]])
        nacts.append(nact)

    SAC = int(max((n + 127) // 128 for n in nacts))
    SA = SAC * 128

    # tile structure in (sorted-key, permuted-q) space, unioned over batches
    live = np.zeros((SAC, 8), dtype=bool)
    full = np.ones((SAC, 8), dtype=bool)
    for b in range(B):
        ps = pos_sels[b]
        n = nacts[b]
        qpos = pos[b][perms[b]]
        qmax = qpos.reshape(8, 128).max(axis=1)
        qmin = qpos.reshape(8, 128).min(axis=1)
        for kc in range(SAC):
            ks, ke = kc * 128, min(kc * 128 + 128, n)
            for qt in range(8):
                if ks >= n:
                    full[kc, qt] = False
                    continue
                l = ps[ks] <= qmax[qt]
                f = (ke - ks == 128) and (ps[ke - 1] <= qmin[qt])
                live[kc, qt] |= l
                if not (l and f):
                    full[kc, qt] = False

    # two live windows per chunk (active-half qt 0-3, inactive-half qt 4-7),
    # each 256-aligned; kc=0 always covers both halves fully (first key is
    # position 0), so it carries the start=True zeroing of the full psum.
    windows = []
    for kc in range(SAC):
        w = []
        act = [qt for qt in range(4) if live[kc, qt]]
        ina = [qt for qt in range(4, 8) if live[kc, qt]]
        if kc == 0:
            w = [(0, 512), (512, 1024)]
        else:
            if act:
                w.append((min(act) * 128 // 256 * 256, 512))
            if ina:
                w.append((512 + (min(ina) - 4) * 128 // 256 * 256, 1024))
        windows.append(tuple(w))
    windows = tuple(windows)

    mask_list = []
    for kc in range(SAC):
        for (s0, s1) in windows[kc]:
            for qt in range(s0 // 128, s1 // 128):
                if not full[kc, qt]:
                    mask_list.append((kc, qt))
    mask_list = sorted(set(mask_list))
    mask_runs = []
    idx = 0
    i = 0
    while i < len(mask_list):
        kc, qt0 = mask_list[i]
        n = 1
        while (i + n < len(mask_list) and mask_list[i + n] == (kc, qt0 + n)):
            n += 1
        mask_runs.append((kc, qt0, n, idx))
        idx += n
        i += n
    mask_runs = tuple(mask_runs)
    meta = (SAC, windows, mask_runs, idx)
    NSTR = max(1, idx)

    s2 = np.concatenate([sin.T[64:], -sin.T[:64]], axis=0)  # [HD, S] table

    def chunked(a, nchunks):
        F = a.shape[1]
        return np.ascontiguousarray(
            a.reshape(nchunks, 128, F).transpose(1, 0, 2).reshape(128, nchunks * F)
        )

    in_maps = []
    for core in range(8):
        b, g = divmod(core, 4)
        n = nacts[b]
        ps = pos_sels[b]
        xperm = hs[b][perms[b]]         # [S, D] rows in permuted-q order
        qpos = pos[b][perms[b]]

        cqb = cos.T[:, qpos]            # rope tables gathered to permuted q
        sqb = s2[:, qpos]

        mk = np.zeros((128, NSTR * 128), dtype=np.float32)
        kidx = np.arange(128)
        for (kc, qt0, nt, idx0) in mask_runs:
            for j in range(nt):
                qt = qt0 + j
                ks = kc * 128
                kvalid = (ks + kidx) < n
                kp = ps[np.minimum(ks + kidx, max(n - 1, 0))]
                qp = qpos[qt * 128:(qt + 1) * 128]
                mk[:, (idx0 + j) * 128:(idx0 + j + 1) * 128] = (
                    kvalid[:, None] & (kp[:, None] <= qp[None, :])
                ).astype(np.float32)

        # wq head-major: head h's 16 chunk-blocks contiguous so each head's
        # Q chain depends on exactly one DMA
        wq_g = wq[:, g * 512:(g + 1) * 512]               # [D, 4*128]
        wq_hm = np.concatenate(
            [chunked(wq_g[:, h * 128:(h + 1) * 128], DC) for h in range(NH)],
            axis=1,
        )

        in_maps.append({
            "xs": chunked(xperm.T.astype(bf16), DC),
            "wq": wq_hm.astype(bf16),
            "wk": chunked(wk[:, g * 128:(g + 1) * 128].astype(bf16), DC),
            "wv": chunked(wv[:, g * 128:(g + 1) * 128].astype(bf16), DC),
            "wo": chunked(wo[g * 512:(g + 1) * 512].astype(bf16), NH),
            "cs": np.concatenate([cqb, sqb], axis=1).astype(bf16),
            "mk": mk.astype(bf16),
        })
    return meta, perms, in_maps


def kernel(hidden_states, cos, sin, wq, wk, wv, wo, position_ids, active_mask):
    global LAST_EXEC_NS, LAST_RESULTS
    from concourse.bass_utils import run_bass_kernel_spmd

    meta, perms, in_maps = _host_prep(
        hidden_states, cos, sin, wq, wk, wv, wo, position_ids, active_mask
    )
    nc = _get_nc(meta)
    res = run_bass_kernel_spmd(nc, in_maps, core_ids=list(range(8)), trace=TRACE)
    LAST_EXEC_NS = res.exec_time_ns
    LAST_RESULTS = res
    B = np.asarray(hidden_states).shape[0]
    full = np.zeros((B, S, D), dtype=np.float32)
    for core in range(8):
        b = core // 4
        o = np.asarray(res.results[core]["out"]).astype(np.float32)
        outT = o.reshape(128, DC, S).transpose(1, 0, 2).reshape(D, S)
        full[b][perms[b]] += outT.T
    return full


# revision 25
# speedup vs baseline: 1.0343x; 1.0343x over previous
"""Trainium2 Bass kernel for MoRAttention (sparse selective-KV GQA attention).

Math: the reference's argsort/gather of active keys == dense attention with
mask = active[k] & (pos[k] <= pos[q]) (softmax is permutation invariant along
keys; q columns are independent). We permute the q axis on the host to
actives-first order, so the first SA columns of the permuted hidden state ARE
the compacted keys: K/V projections read a prefix of the same SBUF tensor the
Q projection reads — no separate gathered-x input. The host inverse-permutes
the output columns.

Causal structure over sorted keys/queries lets us statically skip dead
(k-chunk, q-span) tiles; tiles that are computed but not fully valid get a
multiplicative mask precomputed on the host.

Sharding: 8 cores = 2 batches x 4 kv-groups. Core (b, g) computes q-heads
[4g, 4g+4) + kv-head g of batch b, producing a partial o_proj output
out^T [D, S]; the host sums the 4 partials per batch.

All matmul operands bf16 (same PE rate as f32r, half the DMA/SBUF traffic),
f32 PSUM.

Schedule: inputs stream as ~20 wide DMAs (HWDGE trigger cost ~0.6us each, so
few+large beats many+small) ordered to match consumption; the K, V and Q0
projection chains interleave per-chunk behind the xs stream so the PE starts
as soon as chunk 0 lands. V is computed directly in [key, hd] layout
(lhsT = xs key-block) so no psum transpose is needed. RoPE half-swaps use
partition-shifted DVE multiplies (no SBUF-SBUF DMA). Late-consumed bulk
(wq for heads 2-3, wo) is held behind a dummy dependency so its HBM traffic
does not compete with the critical stream. Attention heads pipeline with the
next head's Q-projection as PE filler; o_proj drains per-half for a short
tail.
"""

import numpy as np

S, D, HD = 1024, 2048, 128
NH = 4           # q heads per core
DC = D // 128    # contraction chunks
SCALE = HD ** -0.5

TRACE = False
DEBUG_TAPS = False
LAST_EXEC_NS = None
LAST_RESULTS = None

_NC_CACHE = {}


def _build_nc(meta):
    import concourse.mybir as mybir
    from concourse import bacc
    from concourse.tile import TileContext
    from contextlib import ExitStack

    SAC, windows, mask_runs, nstr = meta
    SA = SAC * 128
    NSTR = max(1, nstr)
    runs_by_kc = {}
    for (kc, qt0, n, idx0) in mask_runs:
        runs_by_kc.setdefault(kc, []).append((qt0, n, idx0))

    f32 = mybir.dt.float32
    bf16 = mybir.dt.bfloat16
    f8 = mybir.dt.float8e4  # mask values are 0/1 — exact in fp8, half the bytes
    Exp = mybir.ActivationFunctionType.Exp

    nc = bacc.Bacc("TRN2", target_bir_lowering=False, debug=False)

    xs_d = nc.dram_tensor("xs", [128, DC * S], bf16, kind="ExternalInput")
    wq_d = nc.dram_tensor("wq", [128, NH * DC * 128], bf16, kind="ExternalInput")
    wk_d = nc.dram_tensor("wk", [128, DC * 128], bf16, kind="ExternalInput")
    wv_d = nc.dram_tensor("wv", [128, DC * 128], bf16, kind="ExternalInput")
    wo_d = nc.dram_tensor("wo", [128, NH * D], bf16, kind="ExternalInput")
    cs_d = nc.dram_tensor("cs", [128, 2 * S], bf16, kind="ExternalInput")
    mk_d = nc.dram_tensor("mk", [128, NSTR * 128], f8, kind="ExternalInput")
    out_d = nc.dram_tensor("out", [128, DC * S], bf16, kind="ExternalOutput")
    if DEBUG_TAPS:
        dbg_q = [nc.dram_tensor(f"dbg_q{h}", [128, S], bf16, kind="ExternalOutput")
                 for h in range(NH)]
        dbg_kT = nc.dram_tensor("dbg_kT", [128, SA], bf16, kind="ExternalOutput")
        dbg_vn = nc.dram_tensor("dbg_vn", [128, SA], bf16, kind="ExternalOutput")
        dbg_at = [nc.dram_tensor(f"dbg_at{h}", [128, S], bf16, kind="ExternalOutput")
                  for h in range(NH)]
        dbg_c = nc.dram_tensor("dbg_c", [128, S], mybir.dt.float32, kind="ExternalOutput")
        dbg_o = nc.dram_tensor("dbg_o", [128, S], mybir.dt.float32, kind="ExternalOutput")
        dbg_p = nc.dram_tensor("dbg_p", [128, S], bf16, kind="ExternalOutput")

    with TileContext(nc) as tc, ExitStack() as ctx:
        singles = ctx.enter_context(tc.tile_pool(name="singles", bufs=1))
        persist = ctx.enter_context(tc.tile_pool(name="persist", bufs=1))

        ones_tmp = singles.tile([128, 128], f32)
        nc.vector.memset(ones_tmp, 1.0)
        ones128 = singles.tile([128, 128], bf16)
        nc.vector.tensor_copy(ones128, ones_tmp)

        xs_sb = persist.tile([128, DC * S], bf16, tag="xs")
        wq_sb = persist.tile([128, NH * DC * 128], bf16, tag="wq")
        wk_sb = persist.tile([128, DC * 128], bf16, tag="wk")
        wv_sb = persist.tile([128, DC * 128], bf16, tag="wv")
        wo_sb = persist.tile([128, NH * D], bf16, tag="wo")
        cs_sb = persist.tile([128, 2 * S], bf16, tag="cs")
        mk_sb = persist.tile([128, NSTR * 128], f8, tag="mk")
        cq_sb = cs_sb[:, 0:S]
        sq_sb = cs_sb[:, S:2 * S]

        kT = persist.tile([128, SA], bf16, tag="kT")
        vn = persist.tile([128, SA], bf16, tag="vn")
        qT = [persist.tile([128, S], bf16, tag=f"qT{h}", name=f"qT{h}") for h in range(NH)]
        attn = [persist.tile([128, S], bf16, tag=f"attn{h}", name=f"attn{h}") for h in range(NH)]

        # ---- input DMAs: few, wide transfers (trigger cost ~0.6us each on
        # the issuing queue dominates small DMAs). Emission order == issue
        # order == consumption order. wq is head-major so each head's chain
        # depends on one transfer. wq2/wq3/wo are gated behind a dummy read
        # of qT[0] on the gpsimd queue so their bytes don't compete with the
        # critical K/V/Q0 stream for HBM bandwidth.
        hw = [nc.sync, nc.scalar]
        WQH = DC * 128  # per-head wq span
        # DMA queues drain strictly in trigger order, so the trigger sequence
        # IS the arrival sequence: everything is interleaved to land just
        # before its consumer. First arrivals gate the first matmuls and cold
        # DMA runs below steady bandwidth, so the gating pieces are tiny
        # (wk/wv split, xs chunks 0-1 in halves); wq0 streams as 4 pieces
        # matched to the Q0 chain's chunk progress.
        nc.scalar.dma_start(out=wk_sb[:, 0:512], in_=wk_d[:, 0:512])
        nc.scalar.dma_start(out=wv_sb[:, 0:512], in_=wv_d[:, 0:512])
        WQ4 = WQH // 4
        for dc in range(DC):
            if dc < 2:
                for (a, b) in ((0, 512), (512, S)):
                    hw[dc % 2].dma_start(
                        out=xs_sb[:, dc * S + a: dc * S + b],
                        in_=xs_d[:, dc * S + a: dc * S + b],
                    )
            else:
                hw[dc % 2].dma_start(
                    out=xs_sb[:, dc * S:(dc + 1) * S], in_=xs_d[:, dc * S:(dc + 1) * S]
                )
            if dc == 1:  # wq0 piece 0 just after chunks 0-1 (Q0 lags by 2)
                nc.scalar.dma_start(out=wq_sb[:, 0:WQ4], in_=wq_d[:, 0:WQ4])
            elif dc == 2:  # wk/wv tails before K/V reach chunk 4
                nc.scalar.dma_start(out=wk_sb[:, 512:], in_=wk_d[:, 512:])
                nc.scalar.dma_start(out=wv_sb[:, 512:], in_=wv_d[:, 512:])
            elif dc in (4, 7, 10):
                p = {4: 1, 7: 2, 10: 3}[dc]
                nc.scalar.dma_start(
                    out=wq_sb[:, p * WQ4:(p + 1) * WQ4],
                    in_=wq_d[:, p * WQ4:(p + 1) * WQ4],
                )
            elif dc == 12:
                nc.sync.dma_start(out=cs_sb, in_=cs_d[:, :])
        nc.sync.dma_start(out=wq_sb[:, WQH:2 * WQH], in_=wq_d[:, WQH:2 * WQH])
        nc.scalar.dma_start(out=mk_sb, in_=mk_d[:, :])

        def rope(psum, qs, w, dst, pool):
            # dst = psum*cos + rot_half(psum)*sin, via partition-shifted DVE
            # muls (sq rows are pre-arranged + sign-flipped on the host so
            # the shifted product IS rot_half(psum)*sin).
            cos_t = cq_sb[:, qs:qs + w]
            sin_t = sq_sb[:, qs:qs + w]
            pc = pool.tile([128, w], bf16, tag=f"ropec{w}")
            pw = pool.tile([128, w], bf16, tag=f"ropew{w}")
            nc.vector.tensor_mul(pc, psum, cos_t)
            nc.vector.tensor_mul(pw[0:64, :], psum[64:128, :], sin_t[64:128, :])
            nc.vector.tensor_mul(pw[64:128, :], psum[0:64, :], sin_t[0:64, :])
            nc.vector.tensor_add(dst, pc, pw)

        # ================= Stream phase: K, V, Q0 behind the xs stream ====
        with tc.tile_pool(name="pq", bufs=2, space="PSUM") as pq, \
             tc.tile_pool(name="ropeq", bufs=2) as ropeq:
            with tc.tile_pool(name="pkv", bufs=2, space="PSUM") as pkv:
                psum_k = pkv.tile([128, SA], f32, tag="pkv", name="psum_k")
                vn_ps = pkv.tile([128, SA], f32, tag="pkv", name="vn_ps")
                psq0 = pq.tile([128, 512], f32, tag="pq", name="pq0_0")
                psq1 = pq.tile([128, 512], f32, tag="pq", name="pq0_512")

                def q0_mm(dc):
                    for qs, psq in ((0, psq0), (512, psq1)):
                        nc.tensor.matmul(
                            psq,
                            lhsT=wq_sb[:, dc * 128:(dc + 1) * 128],
                            rhs=xs_sb[:, dc * S + qs: dc * S + qs + 512],
                            start=(dc == 0), stop=(dc == DC - 1),
                        )

                LAG = 2
                for dc in range(DC):
                    for (c0, c1) in ((0, 512), (512, SA)):
                        nc.tensor.matmul(
                            psum_k[:, c0:c1],
                            lhsT=wk_sb[:, dc * 128:(dc + 1) * 128],
                            rhs=xs_sb[:, dc * S + c0: dc * S + c1],
                            start=(dc == 0), stop=(dc == DC - 1),
                        )
                    for kc in range(SAC):
                        # start=True clears the has_written bits of the WHOLE
                        # psum bank (4 chunks of 128 f32 per bank): only the
                        # first chunk in each bank may carry it, the others'
                        # first write then overwrites (has_written clear).
                        nc.tensor.matmul(
                            vn_ps[:, kc * 128:(kc + 1) * 128],
                            lhsT=xs_sb[:, dc * S + kc * 128: dc * S + (kc + 1) * 128],
                            rhs=wv_sb[:, dc * 128:(dc + 1) * 128],
                            start=(dc == 0 and kc % 4 == 0), stop=(dc == DC - 1),
                            skip_group_check=True,
                        )
                    if dc >= LAG:
                        q0_mm(dc - LAG)
                for dc in range(DC - LAG, DC):
                    q0_mm(dc)

                # K-ropes first: psum_k[:, 0:512] finalizes at K(dc15)'s first
                # matmul, so kT chunks 0-3 rope while the V/Q0 tail still
                # streams; Q0's ropes (which gate B0) follow as psq finalizes.
                for kc in range(SAC):
                    c0, c1 = kc * 128, (kc + 1) * 128
                    rope(psum_k[:, c0:c1], c0, 128, kT[:, c0:c1], ropeq)
                    nc.scalar.copy(vn[:, c0:c1], vn_ps[:, c0:c1])
                rope(psq0, 0, 512, qT[0][:, 0:512], ropeq)
                rope(psq1, 512, 512, qT[0][:, 512:], ropeq)

            # delayed bulk loads: their HBM traffic must not compete with the
            # critical stream. The scheduler reorders freely absent deps, so
            # gate each DMA with a real WAW hazard: a tiny copy (gated on
            # qT[0], ready at stream end) into the DMA's destination region.
            gate = qT[0][:, 0:2]
            nc.gpsimd.tensor_copy(wq_sb[:, 2 * WQH:2 * WQH + 2], gate)
            nc.gpsimd.dma_start(out=wq_sb[:, 2 * WQH:3 * WQH], in_=wq_d[:, 2 * WQH:3 * WQH])
            nc.gpsimd.tensor_copy(wq_sb[:, 3 * WQH:3 * WQH + 2], gate)
            nc.gpsimd.dma_start(out=wq_sb[:, 3 * WQH:4 * WQH], in_=wq_d[:, 3 * WQH:4 * WQH])
            nc.gpsimd.tensor_copy(wo_sb[:, 0:2], gate)
            nc.gpsimd.dma_start(out=wo_sb, in_=wo_d[:, :])

            # -------- attention heads + next-head Q chains, pipelined ------
            with tc.tile_pool(name="ps", bufs=2, space="PSUM") as ps_p, \
                 tc.tile_pool(name="po", bufs=1, space="PSUM") as po_p, \
                 tc.tile_pool(name="pc", bufs=1, space="PSUM") as pc_p, \
                 tc.tile_pool(name="ppool", bufs=2) as ppool, \
                 tc.tile_pool(name="rpool", bufs=2) as rpool:

                def q_chain_emit(h):
                    """One Q-projection matmul per next(); rope emitted as
                    each half completes so it overlaps the stream."""
                    for qs in (0, 512):
                        psq = pq.tile([128, 512], f32, tag="pq", name=f"pq{h}_{qs}")
                        for dc in range(DC):
                            nc.tensor.matmul(
                                psq,
                                lhsT=wq_sb[:, (h * DC + dc) * 128:(h * DC + dc + 1) * 128],
                                rhs=xs_sb[:, dc * S + qs: dc * S + qs + 512],
                                start=(dc == 0), stop=(dc == DC - 1),
                            )
                            yield None
                        rope(psq, qs, 512, qT[h][:, qs:qs + 512], ropeq)

                # last key-chunk whose windows touch the active q-half: after
                # its reduce, psum_o/psum_c[:, 0:512] are final and half-0 of
                # the normalize can overlap the remaining chunks' matmuls
                K_LAST_ACT = max(
                    (kc for kc in range(SAC)
                     if any(s0 < 512 for (s0, s1) in windows[kc])),
                    default=SAC - 1,
                )

                def b_norm_half(h, psum_o, psum_c, qs):
                    rb = rpool.tile([128, 512], f32, tag="rb", name=f"rb{h}_{qs}")
                    nc.vector.reciprocal_approx_fast(rb, psum_c[:, qs:qs + 512])
                    nc.vector.tensor_mul(
                        attn[h][:, qs:qs + 512], psum_o[:, qs:qs + 512], rb
                    )

                def b_head(h, psum_o, psum_c, filler):
                    def fill(n):
                        for _ in range(n):
                            next(filler, None)

                    def scores_exp(kc):
                        p_sb = ppool.tile([128, S], bf16, tag="p_sb", name=f"p{h}_{kc}")
                        for (s0, s1) in windows[kc]:
                            psum_s = ps_p.tile([128, 512], f32, tag="ps", name=f"ps{h}_{kc}_{s0}")
                            nc.tensor.matmul(
                                psum_s[:, 0:s1 - s0],
                                lhsT=kT[:, kc * 128:(kc + 1) * 128],
                                rhs=qT[h][:, s0:s1],
                                start=True, stop=True,
                            )
                            fill(3)
                            nc.scalar.activation(
                                p_sb[:, s0:s1], psum_s[:, 0:s1 - s0], Exp, scale=SCALE
                            )
                        for (qt0, n, idx0) in runs_by_kc.get(kc, ()):
                            nc.vector.tensor_mul(
                                p_sb[:, qt0 * 128:(qt0 + n) * 128],
                                p_sb[:, qt0 * 128:(qt0 + n) * 128],
                                mk_sb[:, idx0 * 128:(idx0 + n) * 128],
                            )
                        return p_sb

                    def reduce_chunk(kc, p_sb):
                        # kc=0 zeroes the full psum (it covers both halves);
                        # later chunks accumulate arbitrary 256-aligned
                        # sub-windows, so the emission-time group check is
                        # skipped (stop is simulator-only metadata).
                        start = (kc == 0)
                        stop = (kc == SAC - 1)
                        for (s0, s1) in windows[kc]:
                            nc.tensor.matmul(
                                psum_c[:, s0:s1], lhsT=ones128,
                                rhs=p_sb[:, s0:s1], start=start, stop=stop,
                                skip_group_check=True,
                            )
                            fill(2)
                            nc.tensor.matmul(
                                psum_o[:, s0:s1],
                                lhsT=vn[:, kc * 128:(kc + 1) * 128],
                                rhs=p_sb[:, s0:s1], start=start, stop=stop,
                                skip_group_check=True,
                            )
                            fill(2)

                    prev = None
                    fill(10)
                    for kc in range(SAC):
                        p_sb = scores_exp(kc)
                        if DEBUG_TAPS and h == 0 and kc == 0:
                            nc.gpsimd.dma_start(out=dbg_p[:, :], in_=p_sb)
                        fill(3)
                        if prev is not None:
                            reduce_chunk(prev[0], prev[1])
                            if prev[0] == K_LAST_ACT:
                                b_norm_half(h, psum_o, psum_c, 0)
                        prev = (kc, p_sb)
                    reduce_chunk(prev[0], prev[1])
                    if prev[0] == K_LAST_ACT:
                        b_norm_half(h, psum_o, psum_c, 0)
                    fill(64)  # drain any remaining interleaved Q matmuls

                for h in range(NH):
                    psum_o = po_p.tile([128, S], f32, tag="po", name=f"po{h}")
                    psum_c = pc_p.tile([128, S], f32, tag="pc", name=f"pc{h}")
                    filler = q_chain_emit(h + 1) if h + 1 < NH else iter(())
                    b_head(h, psum_o, psum_c, filler)
                    if DEBUG_TAPS and h == 0:
                        dbg_ct = rpool.tile([128, S], f32, tag="dbgc")
                        dbg_ot = rpool.tile([128, S], f32, tag="dbgo")
                        nc.vector.tensor_copy(dbg_ct, psum_c)
                        nc.vector.tensor_copy(dbg_ot, psum_o)
                        nc.gpsimd.dma_start(out=dbg_c[:, :], in_=dbg_ct)
                        nc.gpsimd.dma_start(out=dbg_o[:, :], in_=dbg_ot)
                    b_norm_half(h, psum_o, psum_c, 512)

        if DEBUG_TAPS:
            for h in range(NH):
                nc.sync.dma_start(out=dbg_q[h][:, :], in_=qT[h])
                nc.sync.dma_start(out=dbg_at[h][:, :], in_=attn[h])
            nc.sync.dma_start(out=dbg_kT[:, :], in_=kT)
            nc.sync.dma_start(out=dbg_vn[:, :], in_=vn)

        # ================= Phase C: out^T = wo^T @ attn =================
        with tc.tile_pool(name="poc", bufs=2, space="PSUM") as poc, \
             tc.tile_pool(name="outp", bufs=3) as outp:
            def copy_piece(use_scalar, dst, src):
                if use_scalar:
                    nc.scalar.copy(dst, src)
                else:
                    nc.vector.tensor_copy(dst, src)

            for dc in range(DC):
                oc = poc.tile([128, S], f32, tag="oc", name=f"oc{dc}")
                osb = outp.tile([128, S], bf16, tag="osb", name=f"osb{dc}")
                last = dc == DC - 1
                for qs in (0, 512):
                    for h in range(NH):
                        nc.tensor.matmul(
                            oc[:, qs:qs + 512],
                            lhsT=wo_sb[:, h * D + dc * 128: h * D + (dc + 1) * 128],
                            rhs=attn[h][:, qs:qs + 512],
                            start=(h == 0), stop=(h == NH - 1),
                        )
                    if last and qs == 512:
                        # split the very last piece across both copy engines +
                        # both DMA queues for the shortest drain
                        copy_piece(True, osb[:, 512:768], oc[:, 512:768])
                        copy_piece(False, osb[:, 768:1024], oc[:, 768:1024])
                        nc.sync.dma_start(
                            out=out_d[:, dc * S + 512: dc * S + 768],
                            in_=osb[:, 512:768],
                        )
                        nc.gpsimd.dma_start(
                            out=out_d[:, dc * S + 768:(dc + 1) * S],
                            in_=osb[:, 768:1024],
                        )
                    else:
                        copy_piece((dc + (qs > 0)) % 2 == 0,
                                   osb[:, qs:qs + 512], oc[:, qs:qs + 512])
                        if last:
                            nc.gpsimd.dma_start(
                                out=out_d[:, dc * S: dc * S + 512],
                                in_=osb[:, 0:512],
                            )
                if not last:
                    (nc.sync if dc % 2 == 0 else nc.gpsimd).dma_start(
                        out=out_d[:, dc * S:(dc + 1) * S], in_=osb
                    )

    nc.compile()
    return nc


def _get_nc(meta):
    if meta not in _NC_CACHE:
        _NC_CACHE[meta] = _build_nc(meta)
    return _NC_CACHE[meta]


def _host_prep(hidden_states, cos, sin, wq, wk, wv, wo, position_ids, active_mask):
    import ml_dtypes

    bf16 = ml_dtypes.bfloat16
    hs = np.asarray(hidden_states, dtype=np.float32)
    cos = np.asarray(cos, dtype=np.float32)
    sin = np.asarray(sin, dtype=np.float32)
    wq = np.asarray(wq, dtype=np.float32)
    wk = np.asarray(wk, dtype=np.float32)
    wv = np.asarray(wv, dtype=np.float32)
    wo = np.asarray(wo, dtype=np.float32)
    pos = np.asarray(position_ids).astype(np.int64)
    am = np.asarray(active_mask).astype(bool)
    B = hs.shape[0]
    assert B == 2 and hs.shape[1] == S and hs.shape[2] == D

    ar = np.arange(S)
    perms, pos_sels, nacts = [], [], []
    for b in range(B):
        # actives-first stable order == full q permutation; its prefix is the
        # compacted-key order
        perm = np.argsort(np.where(am[b], ar, ar + S), kind="stable")
        nact = int(am[b].sum())
        perms.append(perm)
        pos_sels.append(pos[b][perm[:nact]])
        nacts.append(nact)

    SAC = int(max((n + 127) // 128 for n in nacts))
    SA = SAC * 128

    # tile structure in (sorted-key, permuted-q) space, unioned over batches
    live = np.zeros((SAC, 8), dtype=bool)
    full = np.ones((SAC, 8), dtype=bool)
    for b in range(B):
        ps = pos_sels[b]
        n = nacts[b]
        qpos = pos[b][perms[b]]
        qmax = qpos.reshape(8, 128).max(axis=1)
        qmin = qpos.reshape(8, 128).min(axis=1)
        for kc in range(SAC):
            ks, ke = kc * 128, min(kc * 128 + 128, n)
            for qt in range(8):
                if ks >= n:
                    full[kc, qt] = False
                    continue
                l = ps[ks] <= qmax[qt]
                f = (ke - ks == 128) and (ps[ke - 1] <= qmin[qt])
                live[kc, qt] |= l
                if not (l and f):
                    full[kc, qt] = False

    # two live windows per chunk (active-half qt 0-3, inactive-half qt 4-7),
    # each 256-aligned; kc=0 always covers both halves fully (first key is
    # position 0), so it carries the start=True zeroing of the full psum.
    windows = []
    for kc in range(SAC):
        w = []
        act = [qt for qt in range(4) if live[kc, qt]]
        ina = [qt for qt in range(4, 8) if live[kc, qt]]
        if kc == 0:
            w = [(0, 512), (512, 1024)]
        else:
            if act:
                w.append((min(act) * 128 // 256 * 256, 512))
            if ina:
                w.append((512 + (min(ina) - 4) * 128 // 256 * 256, 1024))
        windows.append(tuple(w))
    windows = tuple(windows)

    mask_list = []
    for kc in range(SAC):
        for (s0, s1) in windows[kc]:
            for qt in range(s0 // 128, s1 // 128):
                if not full[kc, qt]:
                    mask_list.append((kc, qt))
    mask_list = sorted(set(mask_list))
    mask_runs = []
    idx = 0
    i = 0
    while i < len(mask_list):
        kc, qt0 = mask_list[i]
        n = 1
        while (i + n < len(mask_list) and mask_list[i + n] == (kc, qt0 + n)):
            n += 1
        mask_runs.append((kc, qt0, n, idx))
        idx += n
        i += n
    mask_runs = tuple(mask_runs)
    meta = (SAC, windows, mask_runs, idx)
    NSTR = max(1, idx)

    s2 = np.concatenate([sin.T[64:], -sin.T[:64]], axis=0)  # [HD, S] table

    def chunked(a, nchunks):
        F = a.shape[1]
        return np.ascontiguousarray(
            a.reshape(nchunks, 128, F).transpose(1, 0, 2).reshape(128, nchunks * F)
        )

    in_maps = []
    for core in range(8):
        b, g = divmod(core, 4)
        n = nacts[b]
        ps = pos_sels[b]
        xperm = hs[b][perms[b]]         # [S, D] rows in permuted-q order
        qpos = pos[b][perms[b]]

        cqb = cos.T[:, qpos]            # rope tables gathered to permuted q
        sqb = s2[:, qpos]

        mk = np.zeros((128, NSTR * 128), dtype=np.float32)
        kidx = np.arange(128)
        for (kc, qt0, nt, idx0) in mask_runs:
            for j in range(nt):
                qt = qt0 + j
                ks = kc * 128
                kvalid = (ks + kidx) < n
                kp = ps[np.minimum(ks + kidx, max(n - 1, 0))]
                qp = qpos[qt * 128:(qt + 1) * 128]
                mk[:, (idx0 + j) * 128:(idx0 + j + 1) * 128] = (
                    kvalid[:, None] & (kp[:, None] <= qp[None, :])
                ).astype(np.float32)

        # wq head-major: head h's 16 chunk-blocks contiguous so each head's
        # Q chain depends on exactly one DMA
        wq_g = wq[:, g * 512:(g + 1) * 512]               # [D, 4*128]
        wq_hm = np.concatenate(
            [chunked(wq_g[:, h * 128:(h + 1) * 128], DC) for h in range(NH)],
            axis=1,
        )

        in_maps.append({
            "xs": chunked(xperm.T.astype(bf16), DC),
            "wq": wq_hm.astype(bf16),
            "wk": chunked(wk[:, g * 128:(g + 1) * 128].astype(bf16), DC),
            "wv": chunked(wv[:, g * 128:(g + 1) * 128].astype(bf16), DC),
            "wo": chunked(wo[g * 512:(g + 1) * 512].astype(bf16), NH),
            "cs": np.concatenate([cqb, sqb], axis=1).astype(bf16),
            "mk": mk.astype(ml_dtypes.float8_e4m3),
        })
    return meta, perms, in_maps


def kernel(hidden_states, cos, sin, wq, wk, wv, wo, position_ids, active_mask):
    global LAST_EXEC_NS, LAST_RESULTS
    from concourse.bass_utils import run_bass_kernel_spmd

    meta, perms, in_maps = _host_prep(
        hidden_states, cos, sin, wq, wk, wv, wo, position_ids, active_mask
    )
    nc = _get_nc(meta)
    res = run_bass_kernel_spmd(nc, in_maps, core_ids=list(range(8)), trace=TRACE)
    LAST_EXEC_NS = res.exec_time_ns
    LAST_RESULTS = res
    B = np.asarray(hidden_states).shape[0]
    full = np.zeros((B, S, D), dtype=np.float32)
    for core in range(8):
        b = core // 4
        o = np.asarray(res.results[core]["out"]).astype(np.float32)
        outT = o.reshape(128, DC, S).transpose(1, 0, 2).reshape(D, S)
        full[b][perms[b]] += outT.T
    return full
